# revision 1
# baseline (speedup 1.0000x reference)
"""Multi-head attention kernel for Trainium2 (8 NeuronCores, SPMD).

Sharding: core c handles batch b=c//2 and 4 of the 8 heads
(projection columns 128*(c%2) .. +128).  Each core computes a partial
output projection; the host sums the two partials per batch and adds bo.

v2 structure: the mask is applied multiplicatively AFTER the exp
(w = exp(s) * m, m in {0,1} fp16) so the PE never runs identity-inject
matmuls and the DVE never does fp32 PSUM bias-adds.  The mask tile is
loaded once per (j, kb) and shared by all 4 heads (one 2MB prefetch per
j block).

Per core, S=2048, D=256, 4 heads of dh=32, all hot matmuls fp16:
  qT/kT = (x @ W).T in [proj, S] layout (PE; Act adds bias, fp16 out)
  v     = x @ Wv natural [S, proj] (+ ones col for fused denominator)
  sT[k,q] = sum_d kT[d,k] qT[d,q]  (4 heads row-tiled on the PE)
  w  = exp(sT) on Act (fp16), wm = w * m on DVE (2-byte mode)
  av[d,q] + den[q] = [v|1].T @ wm  (fp32 PSUM accum over kb)
  o  = av * (1/den)  (ones-matmul bcast + reciprocal + mul)
  out_partial[q,:] = sum_h o_h.T @ Wo_h
"""

import numpy as np
import ml_dtypes

import concourse.bass as bass
import concourse.tile as tile
from concourse import bacc, mybir
from concourse.bass_utils import run_bass_kernel_spmd
from concourse._compat import with_exitstack
from contextlib import ExitStack

B, D = 4, 256
H = 8
PROJ = 256
DH = PROJ // H            # 32
NCORES = 8
HPC = H // 2              # heads per core
PC = HPC * DH             # projection cols per core = 128
QB = 512                  # q block (PE moving dim / PSUM bank)
KBK = 128                 # k block

F32 = mybir.dt.float32
F16 = mybir.dt.float16
Identity = mybir.ActivationFunctionType.Identity
Exp = mybir.ActivationFunctionType.Exp
ts = bass.ts


@with_exitstack
def _emit(ctx: ExitStack, tc: tile.TileContext, t: dict, S: int):
    nc = tc.nc
    NQB = S // QB
    NKB = S // KBK

    wt = ctx.enter_context(tc.tile_pool(name="wt", bufs=1))
    sb = ctx.enter_context(tc.tile_pool(name="sb", bufs=1))
    wexp = ctx.enter_context(tc.tile_pool(name="wexp", bufs=3))
    nrm = ctx.enter_context(tc.tile_pool(name="nrm", bufs=2))
    ps = ctx.enter_context(tc.tile_pool(name="ps", bufs=2, space="PSUM"))
    avps = ctx.enter_context(tc.tile_pool(name="avps", bufs=4, space="PSUM"))

    # ---- persistent activations ----
    qT = sb.tile([128, S], F16)          # [proj_col, q]
    kT = sb.tile([128, S], F16)          # [proj_col, k]
    vaug = sb.tile([128, HPC, NKB, 33], F16)  # [k_in_blk, head, k_blk, dh+1]
    oT4 = sb.tile([32, HPC, S], F16)     # per-head attn out, rows 0-31
    m_sb = sb.tile([128, 2, NKB, QB], F16)    # mask prefetch, dbl-buffered

    # ---- constants ----
    wq_s = wt.tile([128, 2, PC], F16)
    wk_s = wt.tile([128, 2, PC], F16)
    wv_s = wt.tile([128, 2, PC], F16)
    for c in range(2):
        nc.sync.dma_start(out=wq_s[:, c, :], in_=t["wq"][ts(c, 128), :])
        nc.sync.dma_start(out=wk_s[:, c, :], in_=t["wk"][ts(c, 128), :])
        nc.sync.dma_start(out=wv_s[:, c, :], in_=t["wv"][ts(c, 128), :])
    bq_s = wt.tile([128, 1], F32)
    bk_s = wt.tile([128, 1], F32)
    nc.sync.dma_start(out=bq_s[:], in_=t["bq"][:, :])
    nc.sync.dma_start(out=bk_s[:], in_=t["bk"][:, :])
    bv_bc = wt.tile([128, PC], F32)
    nc.sync.dma_start(out=bv_bc[:], in_=t["bv"].to_broadcast([128, PC]))
    ones_sb = wt.tile([128, 32], F16)
    nc.sync.dma_start(out=ones_sb[:], in_=t["ones32"][:, :])
    wo4_s = wt.tile([32, HPC, D], F16)
    nc.sync.dma_start(out=wo4_s[:], in_=t["wo4"][:, :, :])
    nc.gpsimd.memset(vaug[:, :, :, 32:33], 1.0)

    # prefetch mask for j=0 right away
    nc.sync.dma_start(
        out=m_sb[:, 0, :, :],
        in_=t["m01"][:, ts(0, QB)].rearrange("(kb p) q -> p kb q", p=128),
    )

    with tc.tile_pool(name="xin", bufs=1) as xin:
        xq_s = xin.tile([128, 2, S], F16)
        xk_s = xin.tile([128, 2, S], F16)
        xv_s = xin.tile([128, 2, S], F16)
        for c in range(2):
            nc.sync.dma_start(out=xq_s[:, c, :], in_=t["xq"][ts(c, 128), :])
            nc.sync.dma_start(out=xk_s[:, c, :], in_=t["xk"][ts(c, 128), :])
            nc.sync.dma_start(out=xv_s[:, c, :], in_=t["xv"][ts(c, 128), :])

        # ---- q/k projections: psum = W.T @ xT  -> [proj, S] ----
        for dst, xs, ws, bs in ((qT, xq_s, wq_s, bq_s), (kT, xk_s, wk_s, bk_s)):
            for j in range(NQB):
                p = ps.tile([128, 2, QB], F32, tag="mm")
                for c in range(2):
                    nc.tensor.matmul(
                        p[:, 0, :],
                        lhsT=ws[:, c, :],
                        rhs=xs[:, c, ts(j, QB)],
                        start=(c == 0),
                        stop=(c == 1),
                    )
                nc.scalar.activation(
                    out=dst[:, ts(j, QB)], in_=p[:, 0, :],
                    func=Identity, bias=bs[:, 0:1], scale=1.0,
                )

        # ---- v projection in natural layout ----
        for sbk in range(NKB):
            p = ps.tile([128, 2, QB], F32, tag="mm")
            for c in range(2):
                nc.tensor.matmul(
                    p[:, 0, 0:PC],
                    lhsT=xv_s[:, c, ts(sbk, 128)],
                    rhs=wv_s[:, c, :],
                    start=(c == 0),
                    stop=(c == 1),
                )
            nc.vector.tensor_add(
                vaug[:, :, sbk, 0:32],
                p[:, 0, 0:PC].rearrange("p (h d) -> p h d", h=HPC),
                bv_bc[:, :].rearrange("p (h d) -> p h d", h=HPC),
            )

    # ---- attention main loop ----
    for j in range(NQB):
        jb = j % 2
        if j + 1 < NQB:
            nc.sync.dma_start(
                out=m_sb[:, (j + 1) % 2, :, :],
                in_=t["m01"][:, ts(j + 1, QB)]
                    .rearrange("(kb p) q -> p kb q", p=128),
            )
        av = [avps.tile([128, QB], F32, tag="av", name=f"av{h}")
              for h in range(HPC)]
        for kb in range(NKB):
            for pair in range(2):
                sc = ps.tile([128, 2, QB], F32, tag="mm")
                for i in range(2):
                    h = 2 * pair + i
                    nc.tensor.matmul(
                        sc[:, i, :],
                        lhsT=kT[32 * h:32 * h + 32, ts(kb, KBK)],
                        rhs=qT[32 * h:32 * h + 32, ts(j, QB)],
                        start=True, stop=True,
                        tile_position=(32 * h, 0),
                    )
                w = wexp.tile([128, 2, QB], F16, tag="w")
                nc.scalar.activation(out=w[:], in_=sc[:], func=Exp)
                wm = wexp.tile([128, 2, QB], F16, tag="wm")
                nc.vector.tensor_mul(
                    wm[:],
                    w[:],
                    m_sb[:, jb, kb, :]
                        .rearrange("p (o n) -> p o n", o=1)
                        .to_broadcast([128, 2, QB]),
                )
                for i in range(2):
                    h = 2 * pair + i
                    nc.tensor.matmul(
                        av[h][0:33, :],
                        lhsT=vaug[:, h, kb, :],
                        rhs=wm[:, i, :],
                        start=(kb == 0),
                        stop=(kb == NKB - 1),
                    )
        # ---- normalize: oT4 rows = av rows * (1/den) ----
        for pair in range(2):
            den = nrm.tile([128, 2, QB], F16, tag="den")
            for i in range(2):
                h = 2 * pair + i
                nc.vector.tensor_copy(out=den[32:33, i, :],
                                      in_=av[h][32:33, :])
            pb = ps.tile([128, 2, QB], F32, tag="mm")
            for i in range(2):
                nc.tensor.matmul(
                    pb[0:32, i, :], lhsT=ones_sb[32:33, :],
                    rhs=den[32:33, i, :],
                    start=True, stop=True, tile_position=(32, 0),
                )
            rec = nrm.tile([32, 2, QB], F32, tag="rec")
            nc.vector.reciprocal_approx_fast(rec[:], pb[0:32, :, :])
            for i in range(2):
                h = 2 * pair + i
                nc.vector.tensor_mul(
                    oT4[0:32, h, ts(j, QB)], av[h][0:32, :], rec[:, i, :],
                )

    # ---- output projection: out[q, :] = sum_h oT_h.T @ Wo_h ----
    for qb in range(S // 128):
        p = ps.tile([128, 2, QB], F32, tag="mm")
        for h in range(HPC):
            nc.tensor.matmul(
                p[:, 0, 0:D],
                lhsT=oT4[0:32, h, ts(qb, 128)],
                rhs=wo4_s[:, h, :],
                start=(h == 0), stop=(h == HPC - 1),
            )
        ob = wexp.tile([128, D], F32, tag="outbuf")
        nc.vector.tensor_copy(out=ob[:], in_=p[:, 0, 0:D])
        nc.sync.dma_start(out=t["out"][ts(qb, 128), :], in_=ob[:])


def build(S: int = 2048):
    nc = bacc.Bacc("TRN2", target_bir_lowering=False, debug=False,
                   num_devices=NCORES)
    t = {}
    t["xq"] = nc.dram_tensor("xq", [D, S], F16, kind="ExternalInput").ap()
    t["xk"] = nc.dram_tensor("xk", [D, S], F16, kind="ExternalInput").ap()
    t["xv"] = nc.dram_tensor("xv", [D, S], F16, kind="ExternalInput").ap()
    t["wq"] = nc.dram_tensor("wq", [D, PC], F16, kind="ExternalInput").ap()
    t["wk"] = nc.dram_tensor("wk", [D, PC], F16, kind="ExternalInput").ap()
    t["wv"] = nc.dram_tensor("wv", [D, PC], F16, kind="ExternalInput").ap()
    t["wo4"] = nc.dram_tensor("wo4", [32, HPC, D], F16,
                              kind="ExternalInput").ap()
    t["ones32"] = nc.dram_tensor("ones32", [128, 32], F16,
                                 kind="ExternalInput").ap()
    t["bq"] = nc.dram_tensor("bq", [PC, 1], F32, kind="ExternalInput").ap()
    t["bk"] = nc.dram_tensor("bk", [PC, 1], F32, kind="ExternalInput").ap()
    t["bv"] = nc.dram_tensor("bv", [1, PC], F32, kind="ExternalInput").ap()
    t["m01"] = nc.dram_tensor("m01", [S, S], F16, kind="ExternalInput").ap()
    t["out"] = nc.dram_tensor("out", [S, D], F32, kind="ExternalOutput").ap()

    with tile.TileContext(nc) as tc:
        _emit(tc, t, S)
    nc.compile()
    return nc


_NC_CACHE = {}


def _get_nc(S):
    if S not in _NC_CACHE:
        _NC_CACHE[S] = build(S)
    return _NC_CACHE[S]


def _pack_wo4(wo_slice):
    """[PC, D] -> [32, HPC, D] per-head rows."""
    w = np.zeros((32, HPC, D), np.float32)
    for h in range(HPC):
        w[:, h, :] = wo_slice[32 * h:32 * h + 32, :]
    return w


def make_in_maps(queries, keys, values, mask, Wq, bq, Wk, bk, Wv, bv, Wo, bo):
    queries = np.asarray(queries, np.float32)
    keys = np.asarray(keys, np.float32)
    values = np.asarray(values, np.float32)
    mask = np.asarray(mask)
    Wq, Wk, Wv, Wo = (np.asarray(a, np.float32) for a in (Wq, Wk, Wv, Wo))
    bq, bk, bv, bo = (np.asarray(a, np.float32) for a in (bq, bk, bv, bo))
    S = queries.shape[1]
    sc = np.float32(1.0) / np.sqrt(np.float32(PROJ))
    f16 = np.float16
    in_maps = []
    for c in range(NCORES):
        b = c // 2
        p0 = PC * (c % 2)
        m01 = mask[b, 0].T.astype(f16)
        im = {
            "xq": np.ascontiguousarray(queries[b].T).astype(f16),
            "xk": np.ascontiguousarray(keys[b].T).astype(f16),
            "xv": np.ascontiguousarray(values[b].T).astype(f16),
            "wq": (Wq[:, p0:p0 + PC] * sc).astype(f16),
            "wk": Wk[:, p0:p0 + PC].astype(f16),
            "wv": Wv[:, p0:p0 + PC].astype(f16),
            "bq": np.ascontiguousarray((bq[p0:p0 + PC] * sc).reshape(PC, 1)),
            "bk": np.ascontiguousarray(bk[p0:p0 + PC].reshape(PC, 1)),
            "bv": np.ascontiguousarray(bv[p0:p0 + PC].reshape(1, PC)),
            "m01": m01,
            "wo4": _pack_wo4(Wo[p0:p0 + PC, :]).astype(f16),
            "ones32": np.ones((128, 32), f16),
        }
        in_maps.append(im)
    return in_maps


def run(inputs, trace=False):
    S = np.asarray(inputs["queries"]).shape[1]
    nc = _get_nc(S)
    in_maps = make_in_maps(**inputs)
    res = run_bass_kernel_spmd(nc, in_maps, core_ids=list(range(NCORES)),
                               trace=trace)
    parts = [np.asarray(r["out"], np.float32) for r in res.results]
    bo = np.asarray(inputs["bo"], np.float32)
    out = np.zeros((B, S, D), np.float32)
    for b in range(B):
        out[b] = parts[2 * b] + parts[2 * b + 1] + bo[None, :]
    return out, res


def kernel(**inputs) -> np.ndarray:
    out, _ = run(inputs, trace=False)
    return out



# revision 13
# speedup vs baseline: 1.1903x; 1.1903x over previous
"""Multi-head attention kernel for Trainium2 (8 NeuronCores, SPMD).

Sharding: core c handles batch b=c//2 and 4 of the 8 heads
(projection columns 128*(c%2) .. +128).  Each core computes a partial
output projection; the host sums the two partials per batch and adds bo.

v3 structure (vs v2 baseline):
  * Score matmuls for all 4 heads are issued back-to-back with
    tile_position=(32h, 0) -> 4-way concurrent row-tiling in the PE
    array (~4x faster scores).
  * AV matmuls are col-tiled 2-way: 33-col stationaries (v | ones) at
    tile_position (0,0) and (0,64), two heads per PSUM bank.
  * The exp+mask work is split between ScalarE (exp, then DVE mask-mul
    in fp16 2x mode) and VectorE (fused masked Schraudolph exp:
    one scalar_tensor_tensor op producing a*exp(s)*m via int16 bit
    tricks), balancing the two engines.
  * Output projection uses Wo as the stationary (contraction 128).
  * fp16 output, transposed [D, S]; host does the final add.

All weights scaled consistently: wm tiles hold a*exp(s)*m with
a = 1477.0 (the mask tiles store {0, a}); the scale cancels in the
softmax normalization.
"""

import numpy as np
import ml_dtypes

import concourse.bass as bass
import concourse.tile as tile
from concourse import bacc, mybir
from concourse.bass_utils import run_bass_kernel_spmd
from concourse._compat import with_exitstack
from contextlib import ExitStack

B, D = 4, 256
H = 8
PROJ = 256
DH = PROJ // H            # 32
NCORES = 8
HPC = H // 2              # heads per core = 4
PC = HPC * DH             # projection cols per core = 128
QB = 512                  # q block
KBK = 128                 # k block

F32 = mybir.dt.float32
F16 = mybir.dt.float16
I16 = mybir.dt.int16
Identity = mybir.ActivationFunctionType.Identity
Exp = mybir.ActivationFunctionType.Exp
ts = bass.ts

# Schraudolph constants: wm = a*exp(s)*m.  Mask stores {0, a}.
A_SCALE = 1477.0
# (s + B_STT) * a  ==  1024*(s*log2e + 15 + log2 a) + C   (C centers the error)
B_STT = (1024.0 * (15.0 + np.log2(A_SCALE)) - 29.5) / A_SCALE

# per-(kb,pair) path pattern: True -> DVE fused exp, False -> ACT exp
# fraction ~0.2 of slices on DVE
def _dve_path(j, kb, pair):
    idx = (j * 16 + kb) * 2 + pair
    return idx % 5 == 0


@with_exitstack
def _emit(ctx: ExitStack, tc: tile.TileContext, t: dict, S: int):
    nc = tc.nc
    NQB = S // QB             # 4
    NKB = S // KBK            # 16

    wt = ctx.enter_context(tc.tile_pool(name="wt", bufs=1))
    sb = ctx.enter_context(tc.tile_pool(name="sb", bufs=1))
    wexp = ctx.enter_context(tc.tile_pool(name="wexp", bufs=3))
    wmp = ctx.enter_context(tc.tile_pool(name="wmp", bufs=3))
    nrm = ctx.enter_context(tc.tile_pool(name="nrm", bufs=2))
    ring = ctx.enter_context(tc.tile_pool(name="ring", bufs=3, space="PSUM"))
    avps = ctx.enter_context(tc.tile_pool(name="avps", bufs=1, space="PSUM"))

    # ---- persistent activations ----
    qT = sb.tile([128, S], F16)          # [proj_col, q]
    kT = sb.tile([128, S], F16)          # [proj_col, k]
    vaug = sb.tile([128, HPC, NKB, 33], F16)  # [k_in_blk, head, k_blk, dh+1]
    oTpA = sb.tile([128, S], F16)        # rows 0-31: h0, 64-95: h1
    oTpB = sb.tile([128, S], F16)        # rows 0-31: h2, 64-95: h3
    m_sb = sb.tile([128, 2, NKB, QB], F16)    # mask {0, a}, dbl-buffered
    den2 = sb.tile([33, 2, QB], F16)     # den rows at partitions 0 and 32

    # ---- constants ----
    wq_s = wt.tile([128, 2, PC], F16)
    wk_s = wt.tile([128, 2, PC], F16)
    wv_s = wt.tile([128, 2, PC], F16)
    bq_s = wt.tile([128, 1], F32)
    bk_s = wt.tile([128, 1], F32)
    bv_bc = wt.tile([128, PC], F32)
    bc2 = wt.tile([33, 128], F16)        # den-broadcast stationary
    woA = wt.tile([128, 2, 128], F16)    # Wo stack for oTpA (2 d-chunks)
    woB = wt.tile([128, 2, 128], F16)
    warm = wt.tile([128, 1], F32)

    nc.sync.dma_start(out=bc2[:], in_=t["bc2"][:, :])
    # warm up the exp table set ASAP (overlaps input DMAs)
    nc.gpsimd.memset(warm[:], 0.0)
    nc.scalar.activation(out=warm[:], in_=warm[:], func=Exp)

    for c in range(2):
        nc.sync.dma_start(out=wk_s[:, c, :], in_=t["wk"][ts(c, 128), :])
        nc.sync.dma_start(out=wv_s[:, c, :], in_=t["wv"][ts(c, 128), :])
        nc.sync.dma_start(out=wq_s[:, c, :], in_=t["wq"][ts(c, 128), :])
    nc.sync.dma_start(out=bq_s[:], in_=t["bq"][:, :])
    nc.sync.dma_start(out=bk_s[:], in_=t["bk"][:, :])
    nc.sync.dma_start(out=bv_bc[:], in_=t["bv"].to_broadcast([128, PC]))
    nc.sync.dma_start(out=woA[:], in_=t["woA"][:, :].rearrange("p (o n) -> p o n", o=2))
    nc.sync.dma_start(out=woB[:], in_=t["woB"][:, :].rearrange("p (o n) -> p o n", o=2))
    nc.gpsimd.memset(vaug[:, :, :, 32:33], 1.0)
    nc.gpsimd.memset(oTpA[32:64, :], 0.0)
    nc.gpsimd.memset(oTpA[96:128, :], 0.0)
    nc.gpsimd.memset(oTpB[32:64, :], 0.0)
    nc.gpsimd.memset(oTpB[96:128, :], 0.0)
    nc.gpsimd.memset(den2[:, :, :], 0.0)

    # prefetch mask for j=0 in 4 chunks (fine-grained so kb 0 starts early)
    for mc in range(4):
        nc.sync.dma_start(
            out=m_sb[:, 0, ts(mc, 4), :],
            in_=t["m01"][ts(mc, 4 * 128), ts(0, QB)]
                .rearrange("(kb p) q -> p kb q", p=128),
        )

    with tc.tile_pool(name="xin", bufs=1) as xin:
        xq_s = xin.tile([128, 2, S], F16)
        xk_s = xin.tile([128, 2, S], F16)
        xv_s = xin.tile([128, 2, S], F16)
        for c in range(2):
            nc.sync.dma_start(out=xk_s[:, c, :], in_=t["xk"][ts(c, 128), :])
        for c in range(2):
            nc.sync.dma_start(out=xv_s[:, c, :], in_=t["xv"][ts(c, 128), :])
        for c in range(2):
            nc.sync.dma_start(out=xq_s[:, c, :], in_=t["xq"][ts(c, 128), :])

        # ---- k/q projections: psum = W.T @ xT  -> [proj, S] ----
        for dst, xs, ws, bs in ((kT, xk_s, wk_s, bk_s), (qT, xq_s, wq_s, bq_s)):
            for j in range(NQB):
                p = ring.tile([128, 2, QB], F32, tag="sc")
                for c in range(2):
                    nc.tensor.matmul(
                        p[:, 0, :],
                        lhsT=ws[:, c, :],
                        rhs=xs[:, c, ts(j, QB)],
                        start=(c == 0),
                        stop=(c == 1),
                    )
                nc.scalar.activation(
                    out=dst[:, ts(j, QB)], in_=p[:, 0, :],
                    func=Identity, bias=bs[:, 0:1], scale=1.0,
                )

        # ---- v projection in natural layout ----
        for sbk in range(NKB):
            p = ring.tile([128, 2, QB], F32, tag="sc")
            for c in range(2):
                nc.tensor.matmul(
                    p[:, 0, 0:PC],
                    lhsT=xv_s[:, c, ts(sbk, 128)],
                    rhs=wv_s[:, c, :],
                    start=(c == 0),
                    stop=(c == 1),
                )
            nc.vector.tensor_add(
                vaug[:, :, sbk, 0:32],
                p[:, 0, 0:PC].rearrange("p (h d) -> p h d", h=HPC),
                bv_bc[:, :].rearrange("p (h d) -> p h d", h=HPC),
            )

    # ---- attention main loop ----
    for j in range(NQB):
        jb = j % 2
        if j + 1 < NQB:
            for mc in range(4):
                nc.sync.dma_start(
                    out=m_sb[:, (j + 1) % 2, ts(mc, 4), :],
                    in_=t["m01"][ts(mc, 4 * 128), ts(j + 1, QB)]
                        .rearrange("(kb p) q -> p kb q", p=128),
                )
        av = avps.tile([128, 2, QB], F32, tag="av")  # bank0: h0/h1, bank1: h2/h3
        for kb in range(NKB):
            sc = [ring.tile([128, 2, QB], F32, tag="sc", name=f"sc{kb}_{p}")
                  for p in range(2)]
            # 4-way row-packed score matmuls
            for h in range(HPC):
                nc.tensor.matmul(
                    sc[h // 2][:, h % 2, :],
                    lhsT=kT[32 * h:32 * h + 32, ts(kb, KBK)],
                    rhs=qT[32 * h:32 * h + 32, ts(j, QB)],
                    start=True, stop=True,
                    tile_position=(32 * h, 0),
                )
            mask_b = (m_sb[:, jb, kb, :]
                      .rearrange("p (o n) -> p o n", o=1)
                      .to_broadcast([128, 2, QB]))
            wm = [None, None]
            for p in range(2):
                wm[p] = wmp.tile([128, 2, QB], F16, tag="wm", name=f"wm{p}")
                if _dve_path(j, kb, p):
                    # fused masked Schraudolph exp on DVE
                    nc.vector.scalar_tensor_tensor(
                        out=wm[p][:].bitcast(I16),
                        in0=sc[p][:],
                        scalar=float(B_STT),
                        in1=mask_b,
                        op0=mybir.AluOpType.add,
                        op1=mybir.AluOpType.mult,
                    )
                else:
                    w = wexp.tile([128, 2, QB], F16, tag="w")
                    nc.scalar.activation(out=w[:], in_=sc[p][:], func=Exp)
                    nc.vector.tensor_mul(wm[p][:], w[:], mask_b)
            # AV matmuls: 2-way col-tiled per bank
            for h in range(HPC):
                bank, pos = h // 2, h % 2
                nc.tensor.matmul(
                    av[64 * pos:64 * pos + 33, bank, :],
                    lhsT=vaug[:, h, kb, :],
                    rhs=wm[bank][:, pos, :],
                    start=(kb == 0),
                    stop=(kb == NKB - 1),
                    tile_position=(0, 64 * pos),
                )
        # ---- normalize: oTp rows = av rows * (1/den) ----
        for bank, oTp in ((0, oTpA), (1, oTpB)):
            # den ~ 1.5e6 overflows fp16: scale by 2^-10 (host compensates)
            nc.vector.tensor_scalar_mul(den2[0:1, bank, :], av[32:33, bank, :],
                                        2.0 ** -10)
            nc.vector.tensor_scalar_mul(den2[32:33, bank, :], av[96:97, bank, :],
                                        2.0 ** -10)
            pbc = ring.tile([128, 2, QB], F32, tag="sc")
            nc.tensor.matmul(
                pbc[:, 0, :], lhsT=bc2[:, :], rhs=den2[:, bank, :],
                start=True, stop=True,
            )
            rec = nrm.tile([128, QB], F32, tag="rec")
            nc.vector.reciprocal_approx_fast(rec[:], pbc[:, 0, :])
            for pos in range(2):
                nc.vector.tensor_mul(
                    oTp[64 * pos:64 * pos + 32, ts(j, QB)],
                    av[64 * pos:64 * pos + 32, bank, :],
                    rec[64 * pos:64 * pos + 32, :],
                )

    # ---- output projection: outT[d, q] = sum oTp rows via Wo stacks ----
    for dc in range(2):
        for qh in range(S // 1024):
            p = ring.tile([128, 2, QB], F32, tag="sc")
            for o in range(2):
                q0 = qh * 1024 + o * QB
                nc.tensor.matmul(
                    p[:, o, :], lhsT=woA[:, dc, :],
                    rhs=oTpA[:, q0:q0 + QB],
                    start=True, stop=False,
                )
                nc.tensor.matmul(
                    p[:, o, :], lhsT=woB[:, dc, :],
                    rhs=oTpB[:, q0:q0 + QB],
                    start=False, stop=True,
                )
            ob = wexp.tile([128, 2, QB], F16, tag="outbuf")
            nc.scalar.activation(out=ob[:], in_=p[:], func=Identity)
            nc.sync.dma_start(
                out=t["out"][ts(dc, 128), ts(qh, 1024)],
                in_=ob[:].rearrange("p o n -> p (o n)"),
            )


def build(S: int = 2048):
    nc = bacc.Bacc("TRN2", target_bir_lowering=False, debug=False,
                   num_devices=NCORES)
    t = {}
    t["xq"] = nc.dram_tensor("xq", [D, S], F16, kind="ExternalInput").ap()
    t["xk"] = nc.dram_tensor("xk", [D, S], F16, kind="ExternalInput").ap()
    t["xv"] = nc.dram_tensor("xv", [D, S], F16, kind="ExternalInput").ap()
    t["wq"] = nc.dram_tensor("wq", [D, PC], F16, kind="ExternalInput").ap()
    t["wk"] = nc.dram_tensor("wk", [D, PC], F16, kind="ExternalInput").ap()
    t["wv"] = nc.dram_tensor("wv", [D, PC], F16, kind="ExternalInput").ap()
    t["woA"] = nc.dram_tensor("woA", [128, D], F16, kind="ExternalInput").ap()
    t["woB"] = nc.dram_tensor("woB", [128, D], F16, kind="ExternalInput").ap()
    t["bc2"] = nc.dram_tensor("bc2", [33, 128], F16, kind="ExternalInput").ap()
    t["bq"] = nc.dram_tensor("bq", [PC, 1], F32, kind="ExternalInput").ap()
    t["bk"] = nc.dram_tensor("bk", [PC, 1], F32, kind="ExternalInput").ap()
    t["bv"] = nc.dram_tensor("bv", [1, PC], F32, kind="ExternalInput").ap()
    t["m01"] = nc.dram_tensor("m01", [S, S], F16, kind="ExternalInput").ap()
    t["out"] = nc.dram_tensor("out", [D, S], F16, kind="ExternalOutput").ap()

    with tile.TileContext(nc) as tc:
        _emit(tc, t, S)
    nc.compile()
    return nc


_NC_CACHE = {}


def _get_nc(S):
    if S not in _NC_CACHE:
        _NC_CACHE[S] = build(S)
    return _NC_CACHE[S]


def make_in_maps(queries, keys, values, mask, Wq, bq, Wk, bk, Wv, bv, Wo, bo):
    queries = np.asarray(queries, np.float32)
    keys = np.asarray(keys, np.float32)
    values = np.asarray(values, np.float32)
    mask = np.asarray(mask)
    Wq, Wk, Wv, Wo = (np.asarray(a, np.float32) for a in (Wq, Wk, Wv, Wo))
    bq, bk, bv, bo = (np.asarray(a, np.float32) for a in (bq, bk, bv, bo))
    S = queries.shape[1]
    sc = np.float32(1.0) / np.sqrt(np.float32(PROJ))
    f16 = np.float16

    # den-broadcast stationary: row0 -> cols 0-31, row32 -> cols 64-95
    bc2 = np.zeros((33, 128), f16)
    bc2[0, 0:32] = 1.0
    bc2[32, 64:96] = 1.0

    in_maps = []
    for c in range(NCORES):
        b = c // 2
        p0 = PC * (c % 2)
        m01 = (mask[b, 0].T.astype(np.float32) * A_SCALE).astype(f16)
        # Wo stacks: oTpA rows 0-31 <- local head 0, 64-95 <- head 1
        #            oTpB rows 0-31 <- local head 2, 64-95 <- head 3
        woA = np.zeros((128, D), np.float32)
        woB = np.zeros((128, D), np.float32)
        woA[0:32] = Wo[p0 + 0 * 32: p0 + 1 * 32, :]
        woA[64:96] = Wo[p0 + 1 * 32: p0 + 2 * 32, :]
        woB[0:32] = Wo[p0 + 2 * 32: p0 + 3 * 32, :]
        woB[64:96] = Wo[p0 + 3 * 32: p0 + 4 * 32, :]
        im = {
            "xq": np.ascontiguousarray(queries[b].T).astype(f16),
            "xk": np.ascontiguousarray(keys[b].T).astype(f16),
            "xv": np.ascontiguousarray(values[b].T).astype(f16),
            "wq": (Wq[:, p0:p0 + PC] * sc).astype(f16),
            "wk": Wk[:, p0:p0 + PC].astype(f16),
            "wv": Wv[:, p0:p0 + PC].astype(f16),
            "bq": np.ascontiguousarray((bq[p0:p0 + PC] * sc).reshape(PC, 1)),
            "bk": np.ascontiguousarray(bk[p0:p0 + PC].reshape(PC, 1)),
            "bv": np.ascontiguousarray(bv[p0:p0 + PC].reshape(1, PC)),
            "m01": m01,
            "woA": woA.astype(f16),
            "woB": woB.astype(f16),
            "bc2": bc2,
        }
        in_maps.append(im)
    return in_maps


def run(inputs, trace=False):
    S = np.asarray(inputs["queries"]).shape[1]
    nc = _get_nc(S)
    in_maps = make_in_maps(**inputs)
    res = run_bass_kernel_spmd(nc, in_maps, core_ids=list(range(NCORES)),
                               trace=trace)
    parts = [np.asarray(r["out"], np.float32) for r in res.results]
    bo = np.asarray(inputs["bo"], np.float32)
    out = np.zeros((B, S, D), np.float32)
    for b in range(B):
        out[b] = (parts[2 * b] + parts[2 * b + 1]).T * np.float32(2.0 ** -10) \
            + bo[None, :]
    return out, res


def kernel(**inputs) -> np.ndarray:
    out, _ = run(inputs, trace=False)
    return out


# revision 19
# speedup vs baseline: 1.2465x; 1.0473x over previous
"""Multi-head attention kernel for Trainium2 (8 NeuronCores, SPMD).

Sharding: core c handles batch b=c//2 and 4 of the 8 heads
(projection columns 128*(c%2) .. +128).  Each core computes a partial
output projection; the host sums the two partials per batch and adds bo.

v3 structure (vs v2 baseline):
  * Score matmuls for all 4 heads are issued back-to-back with
    tile_position=(32h, 0) -> 4-way concurrent row-tiling in the PE
    array (~4x faster scores).
  * AV matmuls are col-tiled 2-way: 33-col stationaries (v | ones) at
    tile_position (0,0) and (0,64), two heads per PSUM bank.
  * The exp+mask work is split between ScalarE (exp, then DVE mask-mul
    in fp16 2x mode) and VectorE (fused masked Schraudolph exp:
    one scalar_tensor_tensor op producing a*exp(s)*m via int16 bit
    tricks), balancing the two engines.
  * Output projection uses Wo as the stationary (contraction 128).
  * fp16 output, transposed [D, S]; host does the final add.

All weights scaled consistently: wm tiles hold a*exp(s)*m with
a = 1477.0 (the mask tiles store {0, a}); the scale cancels in the
softmax normalization.
"""

import numpy as np
import ml_dtypes

import concourse.bass as bass
import concourse.tile as tile
from concourse import bacc, mybir
from concourse.bass_utils import run_bass_kernel_spmd
from concourse._compat import with_exitstack
from contextlib import ExitStack

B, D = 4, 256
H = 8
PROJ = 256
DH = PROJ // H            # 32
NCORES = 8
HPC = H // 2              # heads per core = 4
PC = HPC * DH             # projection cols per core = 128
QB = 512                  # q block
KBK = 128                 # k block

F32 = mybir.dt.float32
F16 = mybir.dt.float16
I16 = mybir.dt.int16
Identity = mybir.ActivationFunctionType.Identity
Exp = mybir.ActivationFunctionType.Exp
ts = bass.ts

# Schraudolph constants: wm = a*exp(s)*m.  Mask stores {0, a}.
A_SCALE = 1477.0
# (s + B_STT) * a  ==  1024*(s*log2e + 15 + log2 a) + C   (C centers the error)
B_STT = (1024.0 * (15.0 + np.log2(A_SCALE)) - 29.5) / A_SCALE

# per-(kb,pair) path pattern: True -> DVE fused exp, False -> ACT exp
# fraction ~0.2 of slices on DVE
def _dve_path(j, kb, pair):
    idx = (j * 16 + kb) * 2 + pair
    return False


@with_exitstack
def _emit(ctx: ExitStack, tc: tile.TileContext, t: dict, S: int):
    nc = tc.nc
    NQB = S // QB             # 4
    NKB = S // KBK            # 16

    wt = ctx.enter_context(tc.tile_pool(name="wt", bufs=1))
    sb = ctx.enter_context(tc.tile_pool(name="sb", bufs=1))
    wexp = ctx.enter_context(tc.tile_pool(name="wexp", bufs=3))
    wmp = ctx.enter_context(tc.tile_pool(name="wmp", bufs=3))
    nrm = ctx.enter_context(tc.tile_pool(name="nrm", bufs=2))
    ring = ctx.enter_context(tc.tile_pool(name="ring", bufs=3, space="PSUM"))
    avps = ctx.enter_context(tc.tile_pool(name="avps", bufs=1, space="PSUM"))

    # ---- persistent activations ----
    qT = sb.tile([128, S], F16)          # [proj_col, q]
    kT = sb.tile([128, S], F16)          # [proj_col, k]
    vaug = sb.tile([128, HPC, NKB, 33], F16)  # [k_in_blk, head, k_blk, dh+1]
    oTpA = sb.tile([128, S], F16)        # rows 0-31: h0, 64-95: h1
    oTpB = sb.tile([128, S], F16)        # rows 0-31: h2, 64-95: h3
    m_sb = sb.tile([128, 2, NKB, QB], F16)    # mask {0, a}, dbl-buffered
    den2 = sb.tile([33, 2, QB], F16)     # den rows at partitions 0 and 32

    # ---- constants ----
    wq_s = wt.tile([128, 2, PC], F16)
    wk_s = wt.tile([128, 2, PC], F16)
    wv_s = wt.tile([128, 2, PC], F16)
    bq_s = wt.tile([128, 1], F32)
    bk_s = wt.tile([128, 1], F32)
    bv_bc = wt.tile([128, PC], F32)
    bc2 = wt.tile([33, 128], F16)        # den-broadcast stationary
    woA = wt.tile([128, 2, 128], F16)    # Wo stack for oTpA (2 d-chunks)
    woB = wt.tile([128, 2, 128], F16)
    warm = wt.tile([128, 1], F32)

    nc.sync.dma_start(out=bc2[:], in_=t["bc2"][:, :])
    # warm up the exp table set ASAP (overlaps input DMAs)
    nc.gpsimd.memset(warm[:], 0.0)
    nc.scalar.activation(out=warm[:], in_=warm[:], func=Exp)

    for c in range(2):
        nc.sync.dma_start(out=wk_s[:, c, :], in_=t["wk"][ts(c, 128), :])
        nc.sync.dma_start(out=wv_s[:, c, :], in_=t["wv"][ts(c, 128), :])
        nc.sync.dma_start(out=wq_s[:, c, :], in_=t["wq"][ts(c, 128), :])
    nc.sync.dma_start(out=bq_s[:], in_=t["bq"][:, :])
    nc.sync.dma_start(out=bk_s[:], in_=t["bk"][:, :])
    nc.sync.dma_start(out=bv_bc[:], in_=t["bv"].to_broadcast([128, PC]))
    nc.sync.dma_start(out=woA[:], in_=t["woA"][:, :].rearrange("p (o n) -> p o n", o=2))
    nc.sync.dma_start(out=woB[:], in_=t["woB"][:, :].rearrange("p (o n) -> p o n", o=2))
    nc.gpsimd.memset(vaug[:, :, :, 32:33], 1.0)
    nc.gpsimd.memset(oTpA[32:64, :], 0.0)
    nc.gpsimd.memset(oTpA[96:128, :], 0.0)
    nc.gpsimd.memset(oTpB[32:64, :], 0.0)
    nc.gpsimd.memset(oTpB[96:128, :], 0.0)
    nc.gpsimd.memset(den2[:, :, :], 0.0)

    # prefetch mask for j=0 in 4 chunks (fine-grained so kb 0 starts early)
    for mc in range(4):
        nc.sync.dma_start(
            out=m_sb[:, 0, ts(mc, 4), :],
            in_=t["m01"][ts(mc, 4 * 128), ts(0, QB)]
                .rearrange("(kb p) q -> p kb q", p=128),
        )

    with tc.tile_pool(name="xin", bufs=1) as xin:
        xq_s = xin.tile([128, 2, S], F16)
        xk_s = xin.tile([128, 2, S], F16)
        xv_s = xin.tile([128, 2, S], F16)
        for c in range(2):
            nc.sync.dma_start(out=xk_s[:, c, :], in_=t["xk"][ts(c, 128), :])
        for c in range(2):
            nc.sync.dma_start(out=xv_s[:, c, :], in_=t["xv"][ts(c, 128), :])
        for c in range(2):
            nc.sync.dma_start(out=xq_s[:, c, :], in_=t["xq"][ts(c, 128), :])

        # ---- k/q projections: psum = W.T @ xT  -> [proj, S] ----
        for dst, xs, ws, bs in ((kT, xk_s, wk_s, bk_s), (qT, xq_s, wq_s, bq_s)):
            for j in range(NQB):
                p = ring.tile([128, 2, QB], F32, tag="sc")
                for c in range(2):
                    nc.tensor.matmul(
                        p[:, 0, :],
                        lhsT=ws[:, c, :],
                        rhs=xs[:, c, ts(j, QB)],
                        start=(c == 0),
                        stop=(c == 1),
                    )
                nc.scalar.activation(
                    out=dst[:, ts(j, QB)], in_=p[:, 0, :],
                    func=Identity, bias=bs[:, 0:1], scale=1.0,
                )

        # ---- v projection in natural layout ----
        for sbk in range(NKB):
            p = ring.tile([128, 2, QB], F32, tag="sc")
            for c in range(2):
                nc.tensor.matmul(
                    p[:, 0, 0:PC],
                    lhsT=xv_s[:, c, ts(sbk, 128)],
                    rhs=wv_s[:, c, :],
                    start=(c == 0),
                    stop=(c == 1),
                )
            nc.vector.tensor_add(
                vaug[:, :, sbk, 0:32],
                p[:, 0, 0:PC].rearrange("p (h d) -> p h d", h=HPC),
                bv_bc[:, :].rearrange("p (h d) -> p h d", h=HPC),
            )

    # ---- attention main loop ----
    for j in range(NQB):
        jb = j % 2
        if j + 1 < NQB:
            for mc in range(4):
                nc.sync.dma_start(
                    out=m_sb[:, (j + 1) % 2, ts(mc, 4), :],
                    in_=t["m01"][ts(mc, 4 * 128), ts(j + 1, QB)]
                        .rearrange("(kb p) q -> p kb q", p=128),
                )
        av = avps.tile([128, 2, QB], F32, tag="av")  # bank0: h0/h1, bank1: h2/h3
        for kb in range(NKB):
            sc = [ring.tile([128, 2, QB], F32, tag="sc", name=f"sc{kb}_{p}")
                  for p in range(2)]
            # 4-way row-packed score matmuls
            for h in range(HPC):
                nc.tensor.matmul(
                    sc[h // 2][:, h % 2, :],
                    lhsT=kT[32 * h:32 * h + 32, ts(kb, KBK)],
                    rhs=qT[32 * h:32 * h + 32, ts(j, QB)],
                    start=True, stop=True,
                    tile_position=(32 * h, 0),
                )
            mask_b = (m_sb[:, jb, kb, :]
                      .rearrange("p (o n) -> p o n", o=1)
                      .to_broadcast([128, 2, QB]))
            wm = [None, None]
            for p in range(2):
                wm[p] = wmp.tile([128, 2, QB], F16, tag="wm", name=f"wm{p}")
                if _dve_path(j, kb, p):
                    # fused masked Schraudolph exp on DVE
                    for i in range(2):
                        nc.vector.scalar_tensor_tensor(
                            out=wm[p][:, i, :].bitcast(I16),
                            in0=sc[p][:, i, :],
                            scalar=float(B_STT),
                            in1=m_sb[:, jb, kb, :],
                            op0=mybir.AluOpType.add,
                            op1=mybir.AluOpType.mult,
                        )
                else:
                    w = wexp.tile([128, 2, QB], F16, tag="w")
                    nc.scalar.activation(out=w[:], in_=sc[p][:], func=Exp)
                    nc.vector.tensor_mul(wm[p][:], w[:], mask_b)
            # AV matmuls: 2-way col-tiled per bank
            for h in range(HPC):
                bank, pos = h // 2, h % 2
                nc.tensor.matmul(
                    av[64 * pos:64 * pos + 33, bank, :],
                    lhsT=vaug[:, h, kb, :],
                    rhs=wm[bank][:, pos, :],
                    start=(kb == 0),
                    stop=(kb == NKB - 1),
                    tile_position=(0, 64 * pos),
                )
        # ---- normalize: oTp rows = av rows * (1/den) ----
        for bank, oTp in ((0, oTpA), (1, oTpB)):
            # den ~ 1.5e6 overflows fp16: scale by 2^-10 (host compensates)
            nc.vector.tensor_scalar_mul(den2[0:1, bank, :], av[32:33, bank, :],
                                        2.0 ** -10)
            nc.vector.tensor_scalar_mul(den2[32:33, bank, :], av[96:97, bank, :],
                                        2.0 ** -10)
            pbc = ring.tile([128, 2, QB], F32, tag="sc")
            nc.tensor.matmul(
                pbc[:, 0, :], lhsT=bc2[:, :], rhs=den2[:, bank, :],
                start=True, stop=True,
            )
            rec = nrm.tile([128, QB], F32, tag="rec")
            nc.vector.reciprocal_approx_fast(rec[:], pbc[:, 0, :])
            for pos in range(2):
                nc.vector.tensor_mul(
                    oTp[64 * pos:64 * pos + 32, ts(j, QB)],
                    av[64 * pos:64 * pos + 32, bank, :],
                    rec[64 * pos:64 * pos + 32, :],
                )

    # ---- output projection: outT[d, q] = sum oTp rows via Wo stacks ----
    for dc in range(2):
        for qh in range(S // 1024):
            p = ring.tile([128, 2, QB], F32, tag="sc")
            for o in range(2):
                q0 = qh * 1024 + o * QB
                nc.tensor.matmul(
                    p[:, o, :], lhsT=woA[:, dc, :],
                    rhs=oTpA[:, q0:q0 + QB],
                    start=True, stop=False,
                )
                nc.tensor.matmul(
                    p[:, o, :], lhsT=woB[:, dc, :],
                    rhs=oTpB[:, q0:q0 + QB],
                    start=False, stop=True,
                )
            ob = wexp.tile([128, 2, QB], F16, tag="outbuf")
            nc.scalar.activation(out=ob[:], in_=p[:], func=Identity)
            nc.sync.dma_start(
                out=t["out"][ts(dc, 128), ts(qh, 1024)],
                in_=ob[:].rearrange("p o n -> p (o n)"),
            )


def build(S: int = 2048):
    nc = bacc.Bacc("TRN2", target_bir_lowering=False, debug=False,
                   num_devices=NCORES)
    t = {}
    t["xq"] = nc.dram_tensor("xq", [D, S], F16, kind="ExternalInput").ap()
    t["xk"] = nc.dram_tensor("xk", [D, S], F16, kind="ExternalInput").ap()
    t["xv"] = nc.dram_tensor("xv", [D, S], F16, kind="ExternalInput").ap()
    t["wq"] = nc.dram_tensor("wq", [D, PC], F16, kind="ExternalInput").ap()
    t["wk"] = nc.dram_tensor("wk", [D, PC], F16, kind="ExternalInput").ap()
    t["wv"] = nc.dram_tensor("wv", [D, PC], F16, kind="ExternalInput").ap()
    t["woA"] = nc.dram_tensor("woA", [128, D], F16, kind="ExternalInput").ap()
    t["woB"] = nc.dram_tensor("woB", [128, D], F16, kind="ExternalInput").ap()
    t["bc2"] = nc.dram_tensor("bc2", [33, 128], F16, kind="ExternalInput").ap()
    t["bq"] = nc.dram_tensor("bq", [PC, 1], F32, kind="ExternalInput").ap()
    t["bk"] = nc.dram_tensor("bk", [PC, 1], F32, kind="ExternalInput").ap()
    t["bv"] = nc.dram_tensor("bv", [1, PC], F32, kind="ExternalInput").ap()
    t["m01"] = nc.dram_tensor("m01", [S, S], F16, kind="ExternalInput").ap()
    t["out"] = nc.dram_tensor("out", [D, S], F16, kind="ExternalOutput").ap()

    with tile.TileContext(nc) as tc:
        _emit(tc, t, S)
    nc.compile()
    return nc


_NC_CACHE = {}


def _get_nc(S):
    if S not in _NC_CACHE:
        _NC_CACHE[S] = build(S)
    return _NC_CACHE[S]


def make_in_maps(queries, keys, values, mask, Wq, bq, Wk, bk, Wv, bv, Wo, bo):
    queries = np.asarray(queries, np.float32)
    keys = np.asarray(keys, np.float32)
    values = np.asarray(values, np.float32)
    mask = np.asarray(mask)
    Wq, Wk, Wv, Wo = (np.asarray(a, np.float32) for a in (Wq, Wk, Wv, Wo))
    bq, bk, bv, bo = (np.asarray(a, np.float32) for a in (bq, bk, bv, bo))
    S = queries.shape[1]
    sc = np.float32(1.0) / np.sqrt(np.float32(PROJ))
    f16 = np.float16

    # den-broadcast stationary: row0 -> cols 0-31, row32 -> cols 64-95
    bc2 = np.zeros((33, 128), f16)
    bc2[0, 0:32] = 1.0
    bc2[32, 64:96] = 1.0

    in_maps = []
    for c in range(NCORES):
        b = c // 2
        p0 = PC * (c % 2)
        m01 = (mask[b, 0].T.astype(np.float32) * A_SCALE).astype(f16)
        # Wo stacks: oTpA rows 0-31 <- local head 0, 64-95 <- head 1
        #            oTpB rows 0-31 <- local head 2, 64-95 <- head 3
        woA = np.zeros((128, D), np.float32)
        woB = np.zeros((128, D), np.float32)
        woA[0:32] = Wo[p0 + 0 * 32: p0 + 1 * 32, :]
        woA[64:96] = Wo[p0 + 1 * 32: p0 + 2 * 32, :]
        woB[0:32] = Wo[p0 + 2 * 32: p0 + 3 * 32, :]
        woB[64:96] = Wo[p0 + 3 * 32: p0 + 4 * 32, :]
        im = {
            "xq": np.ascontiguousarray(queries[b].T).astype(f16),
            "xk": np.ascontiguousarray(keys[b].T).astype(f16),
            "xv": np.ascontiguousarray(values[b].T).astype(f16),
            "wq": (Wq[:, p0:p0 + PC] * sc).astype(f16),
            "wk": Wk[:, p0:p0 + PC].astype(f16),
            "wv": Wv[:, p0:p0 + PC].astype(f16),
            "bq": np.ascontiguousarray((bq[p0:p0 + PC] * sc).reshape(PC, 1)),
            "bk": np.ascontiguousarray(bk[p0:p0 + PC].reshape(PC, 1)),
            "bv": np.ascontiguousarray(bv[p0:p0 + PC].reshape(1, PC)),
            "m01": m01,
            "woA": woA.astype(f16),
            "woB": woB.astype(f16),
            "bc2": bc2,
        }
        in_maps.append(im)
    return in_maps


def run(inputs, trace=False):
    S = np.asarray(inputs["queries"]).shape[1]
    nc = _get_nc(S)
    in_maps = make_in_maps(**inputs)
    res = run_bass_kernel_spmd(nc, in_maps, core_ids=list(range(NCORES)),
                               trace=trace)
    parts = [np.asarray(r["out"], np.float32) for r in res.results]
    bo = np.asarray(inputs["bo"], np.float32)
    out = np.zeros((B, S, D), np.float32)
    for b in range(B):
        out[b] = (parts[2 * b] + parts[2 * b + 1]).T * np.float32(2.0 ** -10) \
            + bo[None, :]
    return out, res


def kernel(**inputs) -> np.ndarray:
    out, _ = run(inputs, trace=False)
    return out


# revision 35
# speedup vs baseline: 1.2477x; 1.0009x over previous
"""Multi-head attention kernel for Trainium2 (8 NeuronCores, SPMD).

Sharding: core c handles batch b=c//2 and 4 of the 8 heads
(projection columns 128*(c%2) .. +128).  Each core computes a partial
output projection; the host sums the two partials per batch and adds bo.

v4 structure:
  * Score matmuls for all 4 heads issued back-to-back with
    tile_position=(32h, 0): 4-way concurrent row-tiling in the PE.
  * AV matmuls col-tiled 2-way: 33-col stationaries (v | ones) at
    tile_position (0,0)/(0,64), two heads per PSUM bank.
  * Pair-granular streaming: score "pair-slices" (2 heads x 512 q)
    stream through a ring of two [128,3,512] PSUM tiles; exp runs at
    FD=1536 per ring tile; mask-muls at fp16 2x; AV matmuls emitted
    per-pair as soon as their weights are ready.
  * Normalize of block j-1 is software-pipelined into block j's first
    two kb's; output projection (Wo stationary, contraction 128) is
    emitted per 1024-q chunk as soon as its oTp columns are final.
  * Input DMAs are sliced per 512-col block and interleaved with the
    projection matmuls so the first scores start within a few us.
"""

import numpy as np
import ml_dtypes

import concourse.bass as bass
import concourse.tile as tile
from concourse import bacc, mybir
from concourse.bass_utils import run_bass_kernel_spmd
from concourse._compat import with_exitstack
from contextlib import ExitStack

B, D = 4, 256
H = 8
PROJ = 256
NCORES = 8
HPC = H // 2              # heads per core = 4
PC = HPC * 32             # projection cols per core = 128
QB = 512                  # q block
KBK = 128                 # k block

F32 = mybir.dt.float32
F16 = mybir.dt.float16
Identity = mybir.ActivationFunctionType.Identity
Exp = mybir.ActivationFunctionType.Exp
ts = bass.ts

A_SCALE = 1477.0          # mask stores {0, a}; cancels in softmax


@with_exitstack
def _emit(ctx: ExitStack, tc: tile.TileContext, t: dict, S: int):
    nc = tc.nc
    NQB = S // QB             # 4
    NKB = S // KBK            # 16

    wt = ctx.enter_context(tc.tile_pool(name="wt", bufs=1))
    sb = ctx.enter_context(tc.tile_pool(name="sb", bufs=1))
    wexp = ctx.enter_context(tc.tile_pool(name="wexp", bufs=3))
    wmp = ctx.enter_context(tc.tile_pool(name="wmp", bufs=3))
    nrm = ctx.enter_context(tc.tile_pool(name="nrm", bufs=2))
    obp = ctx.enter_context(tc.tile_pool(name="obp", bufs=2))
    ring = ctx.enter_context(tc.tile_pool(name="ring", bufs=2, space="PSUM"))
    avps = ctx.enter_context(tc.tile_pool(name="avps", bufs=1, space="PSUM"))

    # ---- persistent activations ----
    qT = sb.tile([128, S], F16)
    kT = sb.tile([128, S], F16)
    vaug = sb.tile([128, HPC, NKB, 33], F16)
    oTpA = sb.tile([128, S], F16)        # rows 0-31: h0, 64-95: h1
    oTpB = sb.tile([128, S], F16)        # rows 0-31: h2, 64-95: h3
    m_sb = sb.tile([128, 2, NKB, QB], F16)
    den2 = sb.tile([33, 2, QB], F16)     # den rows at partitions 0 and 32

    # ---- constants ----
    wq_s = wt.tile([128, 2, PC], F16)
    wk_s = wt.tile([128, 2, PC], F16)
    wv_s = wt.tile([128, 2, PC], F16)
    bq_s = wt.tile([128, 1], F32)
    bk_s = wt.tile([128, 1], F32)
    bv_bc = wt.tile([128, PC], F32)
    bc2 = wt.tile([33, 128], F16)
    woA = wt.tile([128, 2, 128], F16)
    woB = wt.tile([128, 2, 128], F16)
    warm = wt.tile([128, 1], F32)

    # warm up the exp table set ASAP (overlaps input DMAs)
    nc.gpsimd.memset(warm[:], 0.0)
    nc.scalar.activation(out=warm[:], in_=warm[:], func=Exp)

    # weights first (small), then k/q inputs sliced per j block
    for c in range(2):
        nc.sync.dma_start(out=wk_s[:, c, :], in_=t["wk"][ts(c, 128), :])
        nc.sync.dma_start(out=wq_s[:, c, :], in_=t["wq"][ts(c, 128), :])
        nc.sync.dma_start(out=wv_s[:, c, :], in_=t["wv"][ts(c, 128), :])
    nc.sync.dma_start(out=bk_s[:], in_=t["bk"][:, :])
    nc.sync.dma_start(out=bq_s[:], in_=t["bq"][:, :])
    nc.sync.dma_start(out=bv_bc[:], in_=t["bv"].to_broadcast([128, PC]))
    nc.sync.dma_start(out=bc2[:], in_=t["bc2"][:, :])
    nc.sync.dma_start(out=woA[:], in_=t["woA"][:, :].rearrange("p (o n) -> p o n", o=2))
    nc.sync.dma_start(out=woB[:], in_=t["woB"][:, :].rearrange("p (o n) -> p o n", o=2))
    nc.gpsimd.memset(vaug[:, :, :, 32:33], 1.0)
    nc.gpsimd.memset(oTpA[32:64, :], 0.0)
    nc.gpsimd.memset(oTpA[96:128, :], 0.0)
    nc.gpsimd.memset(oTpB[32:64, :], 0.0)
    nc.gpsimd.memset(oTpB[96:128, :], 0.0)
    nc.gpsimd.memset(den2[:, :, :], 0.0)

    xin = ctx.enter_context(tc.tile_pool(name="xin", bufs=1))
    xq_s = xin.tile([128, 2, S], F16)
    xk_s = xin.tile([128, 2, S], F16)
    xv_s = xin.tile([128, 2, S], F16)

    # xk sliced per j; kproj(j) right after its slices
    def proj_j(dst, xs, ws, bs, j):
        p = ring.tile([128, 3, QB], F32, tag="sc", name=f"proj{j}")
        for c in range(2):
            nc.tensor.matmul(
                p[:, 0, :], lhsT=ws[:, c, :], rhs=xs[:, c, ts(j, QB)],
                start=(c == 0), stop=(c == 1),
            )
        nc.vector.tensor_scalar_add(dst[:, ts(j, QB)], p[:, 0, :], bs[:, 0:1])

    for j in range(NQB):
        for c in range(2):
            nc.sync.dma_start(out=xk_s[:, c, ts(j, QB)],
                              in_=t["xk"][ts(c, 128), ts(j, QB)])
        proj_j(kT, xk_s, wk_s, bk_s, j)
    # q j=0 early so scores can begin; mask chunk 0 early too
    for c in range(2):
        nc.sync.dma_start(out=xq_s[:, c, ts(0, QB)],
                          in_=t["xq"][ts(c, 128), ts(0, QB)])
    proj_j(qT, xq_s, wq_s, bq_s, 0)
    nc.sync.dma_start(
        out=m_sb[:, 0, ts(0, 4), :],
        in_=t["m01"][ts(0, 512), ts(0, QB)].rearrange("(kb p) q -> p kb q", p=128),
    )
    # v input + projection (needed by first AV)
    for c in range(2):
        nc.sync.dma_start(out=xv_s[:, c, :], in_=t["xv"][ts(c, 128), :])
    for sbk in range(NKB):
        p = ring.tile([128, 3, QB], F32, tag="sc", name=f"vp{sbk}")
        for c in range(2):
            nc.tensor.matmul(
                p[:, 0, 0:PC], lhsT=xv_s[:, c, ts(sbk, 128)], rhs=wv_s[:, c, :],
                start=(c == 0), stop=(c == 1),
            )
        nc.vector.tensor_add(
            vaug[:, :, sbk, 0:32],
            p[:, 0, 0:PC].rearrange("p (h d) -> p h d", h=HPC),
            bv_bc[:, :].rearrange("p (h d) -> p h d", h=HPC),
        )
    # rest of mask j0 + remaining q projections
    for mc in range(1, 4):
        nc.sync.dma_start(
            out=m_sb[:, 0, ts(mc, 4), :],
            in_=t["m01"][ts(mc, 512), ts(0, QB)]
                .rearrange("(kb p) q -> p kb q", p=128),
        )
    for j in range(1, NQB):
        for c in range(2):
            nc.sync.dma_start(out=xq_s[:, c, ts(j, QB)],
                              in_=t["xq"][ts(c, 128), ts(j, QB)])
        proj_j(qT, xq_s, wq_s, bq_s, j)

    # ---------------- attention: head-granular stream ----------------
    # head-slice g = (j*NKB + kb)*4 + h ; ring tile holds 3 head-slices.
    TILE_P = 3

    state = {
        "tile": None,          # current ring tile being filled
        "w": None,
        "wm": None,
        "fill": 0,             # pairs in current tile
        "tbase": 0,            # g of slice 0
        "ready": [],           # (g, wm_tile, slice) ready for AV
        "av": None,
        "norm_done": -1,       # last j normalized
        "op_done": -1,         # last out-proj chunk emitted
    }

    def flush_tile():
        """emit exp + masked muls + AV for the current ring tile."""
        n = state["fill"]
        if n == 0:
            return
        sc, tb = state["tile"], state["tbase"]
        w = wexp.tile([128, TILE_P, QB], F16, tag="w", name=f"w{tb}")
        nc.scalar.activation(out=w[0:128, 0:n, :], in_=sc[0:128, 0:n, :],
                             func=Exp)
        # masked muls grouped by (j, kb) within the tile
        s = 0
        while s < n:
            g = tb + s
            kb = (g // 4) % NKB
            j = g // (4 * NKB)
            e = s + 1
            while e < n and ((tb + e) // 4) % NKB == kb \
                    and (tb + e) // (4 * NKB) == j:
                e += 1
            wm = wmp.tile([128, TILE_P, QB], F16, tag="wm", name=f"wm{tb}_{s}")
            nc.vector.tensor_mul(
                wm[:, s:e, :],
                w[:, s:e, :],
                m_sb[:, j % 2, kb, :]
                    .rearrange("p (o nn) -> p o nn", o=1)
                    .to_broadcast([128, e - s, QB]),
            )
            for ss in range(s, e):
                state["ready"].append((tb + ss, wm, ss))
            s = e
        state["tile"] = None
        state["fill"] = 0
        drain_ready()

    def do_normalize(j):
        av = state["av"]
        for bank, oTp in ((0, oTpA), (1, oTpB)):
            nc.vector.tensor_scalar_mul(den2[0:1, bank, :],
                                        av[32:33, bank, :], 2.0 ** -10)
            nc.vector.tensor_scalar_mul(den2[32:33, bank, :],
                                        av[96:97, bank, :], 2.0 ** -10)
            pbc = ring.tile([128, 3, QB], F32, tag="sc", name=f"pbc{j}_{bank}")
            nc.tensor.matmul(pbc[:, 0, :], lhsT=bc2[:, :],
                             rhs=den2[:, bank, :], start=True, stop=True)
            rec = nrm.tile([128, QB], F32, tag="rec")
            nc.vector.reciprocal_approx_fast(rec[:], pbc[:, 0, :])
            for pos in range(2):
                nc.vector.tensor_mul(
                    oTp[64 * pos:64 * pos + 32, ts(j, QB)],
                    av[64 * pos:64 * pos + 32, bank, :],
                    rec[64 * pos:64 * pos + 32, :],
                )
        state["norm_done"] = j
        if j == NQB - 1:
            if state["op_done"] < 0:
                emit_outproj(0)
            emit_outproj(1)

    def emit_outproj(qh):
        state["op_done"] = qh
        for dc in range(2):
            p = ring.tile([128, 3, QB], F32, tag="sc", name=f"op{qh}_{dc}")
            for o in range(2):
                q0 = qh * 1024 + o * QB
                nc.tensor.matmul(p[:, o, :], lhsT=woA[:, dc, :],
                                 rhs=oTpA[:, q0:q0 + QB],
                                 start=True, stop=False)
                nc.tensor.matmul(p[:, o, :], lhsT=woB[:, dc, :],
                                 rhs=oTpB[:, q0:q0 + QB],
                                 start=False, stop=True)
            ob = obp.tile([128, 2, QB], F16, tag="outbuf")
            nc.vector.tensor_copy(out=ob[:], in_=p[:, 0:2, :])
            nc.sync.dma_start(
                out=t["out"][ts(dc, 128), ts(qh, 1024)],
                in_=ob[:].rearrange("p o n -> p (o n)"),
            )

    def drain_ready():
        for g, wmt, sl in state["ready"]:
            j = g // (4 * NKB)
            kb = (g // 4) % NKB
            h = g % 4
            if kb == 0 and h == 0:
                state["av"] = avps.tile([128, 2, QB], F32, tag="av",
                                        name=f"av{j}")
            av = state["av"]
            bank, pos = h // 2, h % 2
            nc.tensor.matmul(
                av[64 * pos:64 * pos + 33, bank, :],
                lhsT=vaug[:, h, kb, :],
                rhs=wmt[:, sl, :],
                start=(kb == 0),
                stop=(kb == NKB - 1),
                tile_position=(0, 64 * pos),
            )
        state["ready"] = []

    for j in range(NQB):
        jb = j % 2
        if j + 1 < NQB:
            for mc in range(4):
                nc.sync.dma_start(
                    out=m_sb[:, (j + 1) % 2, ts(mc, 4), :],
                    in_=t["m01"][ts(mc, 512), ts(j + 1, QB)]
                        .rearrange("(kb p) q -> p kb q", p=128),
                )
        for kb in range(NKB):
            for h in range(HPC):
                if state["tile"] is None:
                    g = (j * NKB + kb) * 4 + h
                    state["tile"] = ring.tile([128, TILE_P, QB], F32,
                                              tag="sc", name=f"sc{g}")
                    state["tbase"] = g
                sl = state["fill"]
                nc.tensor.matmul(
                    state["tile"][:, sl, :],
                    lhsT=kT[32 * h:32 * h + 32, ts(kb, KBK)],
                    rhs=qT[32 * h:32 * h + 32, ts(j, QB)],
                    start=True, stop=True,
                    tile_position=(32 * h, 0),
                )
                state["fill"] += 1
                if state["fill"] == TILE_P:
                    flush_tile()
                    if j == 2 and kb >= 8 and state["op_done"] < 0:
                        emit_outproj(0)
        flush_tile()
        do_normalize(j)


def build(S: int = 2048):
    nc = bacc.Bacc("TRN2", target_bir_lowering=False, debug=False,
                   num_devices=NCORES)
    t = {}
    t["xq"] = nc.dram_tensor("xq", [D, S], F16, kind="ExternalInput").ap()
    t["xk"] = nc.dram_tensor("xk", [D, S], F16, kind="ExternalInput").ap()
    t["xv"] = nc.dram_tensor("xv", [D, S], F16, kind="ExternalInput").ap()
    t["wq"] = nc.dram_tensor("wq", [D, PC], F16, kind="ExternalInput").ap()
    t["wk"] = nc.dram_tensor("wk", [D, PC], F16, kind="ExternalInput").ap()
    t["wv"] = nc.dram_tensor("wv", [D, PC], F16, kind="ExternalInput").ap()
    t["woA"] = nc.dram_tensor("woA", [128, D], F16, kind="ExternalInput").ap()
    t["woB"] = nc.dram_tensor("woB", [128, D], F16, kind="ExternalInput").ap()
    t["bc2"] = nc.dram_tensor("bc2", [33, 128], F16, kind="ExternalInput").ap()
    t["bq"] = nc.dram_tensor("bq", [PC, 1], F32, kind="ExternalInput").ap()
    t["bk"] = nc.dram_tensor("bk", [PC, 1], F32, kind="ExternalInput").ap()
    t["bv"] = nc.dram_tensor("bv", [1, PC], F32, kind="ExternalInput").ap()
    t["m01"] = nc.dram_tensor("m01", [S, S], F16, kind="ExternalInput").ap()
    t["out"] = nc.dram_tensor("out", [D, S], F16, kind="ExternalOutput").ap()

    with tile.TileContext(nc) as tc:
        _emit(tc, t, S)
    nc.compile()
    return nc


_NC_CACHE = {}


def _get_nc(S):
    if S not in _NC_CACHE:
        _NC_CACHE[S] = build(S)
    return _NC_CACHE[S]


def make_in_maps(queries, keys, values, mask, Wq, bq, Wk, bk, Wv, bv, Wo, bo):
    queries = np.asarray(queries, np.float32)
    keys = np.asarray(keys, np.float32)
    values = np.asarray(values, np.float32)
    mask = np.asarray(mask)
    Wq, Wk, Wv, Wo = (np.asarray(a, np.float32) for a in (Wq, Wk, Wv, Wo))
    bq, bk, bv, bo = (np.asarray(a, np.float32) for a in (bq, bk, bv, bo))
    S = queries.shape[1]
    sc = np.float32(1.0) / np.sqrt(np.float32(PROJ))
    f16 = np.float16

    bc2 = np.zeros((33, 128), f16)
    bc2[0, 0:32] = 1.0
    bc2[32, 64:96] = 1.0

    in_maps = []
    for c in range(NCORES):
        b = c // 2
        p0 = PC * (c % 2)
        m01 = (mask[b, 0].T.astype(np.float32) * A_SCALE).astype(f16)
        woA = np.zeros((128, D), np.float32)
        woB = np.zeros((128, D), np.float32)
        woA[0:32] = Wo[p0 + 0 * 32: p0 + 1 * 32, :]
        woA[64:96] = Wo[p0 + 1 * 32: p0 + 2 * 32, :]
        woB[0:32] = Wo[p0 + 2 * 32: p0 + 3 * 32, :]
        woB[64:96] = Wo[p0 + 3 * 32: p0 + 4 * 32, :]
        im = {
            "xq": np.ascontiguousarray(queries[b].T).astype(f16),
            "xk": np.ascontiguousarray(keys[b].T).astype(f16),
            "xv": np.ascontiguousarray(values[b].T).astype(f16),
            "wq": (Wq[:, p0:p0 + PC] * sc).astype(f16),
            "wk": Wk[:, p0:p0 + PC].astype(f16),
            "wv": Wv[:, p0:p0 + PC].astype(f16),
            "bq": np.ascontiguousarray((bq[p0:p0 + PC] * sc).reshape(PC, 1)),
            "bk": np.ascontiguousarray(bk[p0:p0 + PC].reshape(PC, 1)),
            "bv": np.ascontiguousarray(bv[p0:p0 + PC].reshape(1, PC)),
            "m01": m01,
            "woA": woA.astype(f16),
            "woB": woB.astype(f16),
            "bc2": bc2,
        }
        in_maps.append(im)
    return in_maps


def run(inputs, trace=False):
    S = np.asarray(inputs["queries"]).shape[1]
    nc = _get_nc(S)
    in_maps = make_in_maps(**inputs)
    res = run_bass_kernel_spmd(nc, in_maps, core_ids=list(range(NCORES)),
                               trace=trace)
    parts = [np.asarray(r["out"], np.float32) for r in res.results]
    bo = np.asarray(inputs["bo"], np.float32)
    out = np.zeros((B, S, D), np.float32)
    for b in range(B):
        out[b] = (parts[2 * b] + parts[2 * b + 1]).T * np.float32(2.0 ** -10) \
            + bo[None, :]
    return out, res


def kernel(**inputs) -> np.ndarray:
    out, _ = run(inputs, trace=False)
    return out
